# revision 20
# baseline (speedup 1.0000x reference)
"""Trainium2 Bass kernel for a 2-layer dense GCN (NodeEncoder).

    out = adj @ relu(adj @ (x@W1) + b1) @ W2 + b2
    N=16384, F_IN=512, HID=1024, OUT=256, adj dense [N, N] fp32.

Sharding: adj row-partitioned across 8 NeuronCores (2048 rows/core).
All device matmuls consume natural-layout (row-major) operands; the
host pre-transposes adj/x per shard so no on-device transposes are
needed.  Per core:

  phase A:  s1_c   = x_c @ W1                  [2048, 1024]  (own rows)
  AG1:      s1     = AllGather(s1_c)           [16384, 1024]
  phase B:  hT_c   = relu(adj_c @ s1 + b1)^T   [1024, 2048]  (transposed
            orientation: lhsT = s1 tiles, rhs = adjT_c tiles -> psum is
            [n, m]; bias b1 is per-partition, fused into the ACT relu)
  phase C:  s2_c   = h_c @ W2                  [2048, 256]   (lhsT = hT_c)
  AG2:      s2     = AllGather(s2_c)           [16384, 256]
  phase D:  out2T_c = (adj_c @ s2)^T + b2      [256, 2048]   (lhsT = s2
            tiles, rhs = adjT_c tiles; b2 per-partition via ACT Copy)

Matmuls run in bf16 with fp32 PSUM accumulation (max rel err vs fp32
reference ~3e-3 of absmax).
"""

import numpy as np
import ml_dtypes

import concourse.bass as bass
import concourse.mybir as mybir
import concourse.tile as tile
from concourse.bass_utils import run_bass_kernel_spmd
from concourse.tile_sem_assignment import N_PROCS
from concourse.vector_clock import ScopedClock, VectorClock
from concourse.tile_rust import add_dep_helper as tile_rust_add_dep

# ---------------------------------------------------------------------------
# Workaround: the walrus build in this container caps the number of sync-wait
# commands on a Drain instruction; Tile's kernel-tail drain aggregates one
# wait per logical processor and exceeds it.  Split the tail drain into a
# chain of single-wait drains on the same (SP) queue — semantically identical.
# ---------------------------------------------------------------------------


def _drain_and_barrier_split(self, tick_clock, wait_clock):
    gc = tick_clock.global_clock
    for p in range(N_PROCS):
        partial = VectorClock([gc[q] if q == p else 0 for q in range(N_PROCS)])
        d = self.nc.sync.drain()
        wait_clock.add_sem_waits(d.ins, ScopedClock({None: partial}))
    self.nc.sync.drain()

    self.nc.all_engine_barrier()
    assert self.sems is not None
    popped = self.nc._tile_sem_poison_stack.pop()
    assert popped is self._sem_poison
    self.nc.clear_and_free_semaphores(list(self.sems.allocated().values()))
    self.nc.all_engine_barrier()


tile.TileContext._drain_and_barrier = _drain_and_barrier_split

# The same walrus cap applies to every instruction kind: at most ONE sync
# wait command per instruction (probed empirically — a 2-wait TensorCopy is
# rejected).  Post-pass: hoist excess sem-waits onto no-ops inserted just
# before the instruction on the same engine queue — per-engine program order
# makes this semantically identical.
_MAX_WAITS = 1


def _split_excess_waits(nc):
    ctr = 0
    for f in nc.m.functions:
        for bb in f.blocks:
            out = []
            changed = False
            for inst in bb.instructions:
                si = inst.sync_info
                waits = list(si.on_wait) if si is not None and si.on_wait else []
                if len(waits) > _MAX_WAITS:
                    changed = True
                    keep, excess = waits[: _MAX_WAITS], waits[_MAX_WAITS :]
                    for i in range(0, len(excess), _MAX_WAITS):
                        ctr += 1
                        nop = mybir.InstNoOp(name=f"I-waitnop-{ctr}")
                        nop.engine = inst.engine
                        nop.sync_info = mybir.SyncInfo(
                            on_wait=excess[i : i + _MAX_WAITS], on_update=[]
                        )
                        out.append(nop)
                    si.on_wait = keep
                out.append(inst)
            if changed:
                bb.instructions = out
    return ctr

def _elide_redundant_ldweights(nc):
    """Delete an InstLdweights that reloads the exact weights AP loaded by
    the previous (surviving) InstLdweights when only plain matmuls / no-ops
    sit between them in the scheduled stream.  The PE array keeps the
    stationary operand across matmuls, so the reload is pure overhead
    (walrus emits one LDWEIGHTS per MATMUL and its ldw-opt pass is
    incompatible with pre-split LDW+MM).  Only sync-free LDWs are removed,
    so semaphore bookkeeping is unchanged."""
    n_elided = 0
    for f in nc.m.functions:
        for bb in f.blocks:
            out = []
            last_w = None  # weights-AP repr of last surviving LDW, if run intact
            changed = False
            for inst in bb.instructions:
                nm = type(inst).__name__
                if nm == "InstLdweights":
                    si = inst.sync_info
                    clean = not (si and (si.on_wait or si.on_update))
                    w = repr(inst.ins[0])
                    if clean and last_w == w:
                        n_elided += 1
                        changed = True
                        continue  # drop the reload
                    last_w = w if clean else None
                elif nm == "InstMatmult":
                    if getattr(inst, "is_transpose", False):
                        last_w = None
                elif nm == "InstNoOp":
                    pass
                else:
                    last_w = None
                out.append(inst)
            if changed:
                bb.instructions = out
    return n_elided


NCORES = 8
N = 16384
SH = N // NCORES  # 2048 adj rows per core
F = 512
HID = 1024
OUT = 256

BF16 = mybir.dt.bfloat16
F32 = mybir.dt.float32

_built = None


def build():
    """Build the per-core Bass program (identical on all cores)."""
    nc = bass.Bass()

    adjT = nc.declare_dram_parameter("adjT", [N, SH], BF16, isOutput=False)
    xT = nc.declare_dram_parameter("xT", [F, SH], BF16, isOutput=False)
    w1 = nc.declare_dram_parameter("w1", [F, HID], BF16, isOutput=False)
    w2 = nc.declare_dram_parameter("w2", [HID, OUT], BF16, isOutput=False)
    b1T = nc.declare_dram_parameter("b1T", [128, HID // 128], F32, isOutput=False)
    b2T = nc.declare_dram_parameter("b2T", [128, OUT // 128], F32, isOutput=False)
    out2T = nc.declare_dram_parameter("out2T", [OUT, SH], F32, isOutput=True)

    rg = [list(range(NCORES))]

    # adjT column-block mb (512 wide), 4 k-blocks per DMA:
    #   [p, k4, kk, m] = adjT[k4*512 + kk*128 + p, mb*512 + m]
    def adjT_src(mb):
        return adjT[:, mb * 512 : (mb + 1) * 512].rearrange(
            "(k4 kk p) m -> p k4 kk m", kk=4, p=128
        )

    def adjTp_src(mbp):
        return adjT[:, mbp * 1024 : (mbp + 1) * 1024].rearrange(
            "(k4 kk p) m -> p k4 kk m", kk=4, p=128
        )

    def allgather(inp, outp):
        return nc.gpsimd.collective_compute(
            "AllGather",
            mybir.AluOpType.bypass,
            replica_groups=rg,
            ins=[inp.opt()],
            outs=[outp.opt()],
        )

    with tile.TileContext(nc) as tc:
        with (
            tc.tile_pool(name="const", bufs=1) as constp,
            tc.tile_pool(name="psum", bufs=8, space="PSUM") as psum,
            tc.tile_pool(name="dram", bufs=1, space="DRAM") as dram,
            tc.tile_pool(name="adj", bufs=4) as adjp,
            tc.tile_pool(name="small", bufs=4) as smallp,
        ):
            # ---- constants ----
            w2t = constp.tile([128, HID // 128, OUT], BF16)
            nc.sync.dma_start(w2t[:], w2[:].rearrange("(f p) n -> p f n", p=128))
            b1t = constp.tile([128, HID // 128], F32)
            nc.sync.dma_start(b1t[:], b1T[:])
            b2t = constp.tile([128, OUT // 128], F32)
            nc.sync.dma_start(b2t[:], b2T[:])

            # AllGathers split in quarters so they overlap compute: phase B
            # can start once the first two s1 quarters have gathered, and
            # phase D streams k-blocks in gather-arrival order.
            ag1h_in = dram.tile([SH, 512], BF16, name="ag1h_in")
            ag1h_out = dram.tile([N, 512], BF16, addr_space="Shared", name="ag1h_out")
            ag1_in = [dram.tile([SH, 256], BF16, name=f"ag1i{q}") for q in (2, 3)]
            ag1_out = [
                dram.tile([N, 256], BF16, addr_space="Shared", name=f"ag1o{q}")
                for q in (2, 3)
            ]
            ag2_in = [dram.tile([SH // 4, OUT], BF16, name=f"ag2i{q}") for q in range(4)]
            ag2_out = [
                dram.tile([N // 4, OUT], BF16, addr_space="Shared", name=f"ag2o{q}")
                for q in range(4)
            ]

            # ---- phase A: s1_c = x_c @ W1 (per n-quarter; AG per quarter) ----
            with tc.tile_pool(name="phA", bufs=1) as pA:
                xt = []
                w1t = []
                for f in range(4):
                    t = pA.tile([128, SH], BF16, name=f"xt{f}")
                    nc.sync.dma_start(t[:], xT[f * 128 : (f + 1) * 128, :])
                    xt.append(t)
                    t = pA.tile([128, HID], BF16, name=f"w1t{f}")
                    nc.sync.dma_start(t[:], w1[f * 128 : (f + 1) * 128, :])
                    w1t.append(t)
                # first n-half in one chunk: it gates phase B's start
                for mt in range(SH // 128):
                    psa = psum.tile([128, 512], F32, tag="ps", name=f"psAh{mt}")
                    for f in range(4):
                        nc.tensor.matmul(
                            psa[:],
                            xt[f][:, mt * 128 : (mt + 1) * 128],
                            w1t[f][:, 0:512],
                            start=(f == 0),
                            stop=(f == 3),
                        )
                    s1o = smallp.tile([128, 512], BF16, tag="s1o", bufs=2)
                    nc.vector.tensor_copy(s1o[:], psa[:])
                    nc.scalar.dma_start(
                        ag1h_in[mt * 128 : (mt + 1) * 128, :], s1o[:]
                    )
                allgather(ag1h_in, ag1h_out)
                # quarters 2,3: computed now, gathered later (delayed deps)
                for qi, q in enumerate((2, 3)):
                    for mt in range(SH // 128):
                        psa = psum.tile([128, 256], F32, tag="ps", name=f"psA{q}{mt}")
                        for f in range(4):
                            nc.tensor.matmul(
                                psa[:],
                                xt[f][:, mt * 128 : (mt + 1) * 128],
                                w1t[f][:, q * 256 : (q + 1) * 256],
                                start=(f == 0),
                                stop=(f == 3),
                            )
                        s1o2 = smallp.tile([128, 256], BF16, tag="s1o2", bufs=2)
                        nc.vector.tensor_copy(s1o2[:], psa[:])
                        nc.scalar.dma_start(
                            ag1_in[qi][mt * 128 : (mt + 1) * 128, :], s1o2[:]
                        )

            # ---- phases B + C (C quarters interleaved so AG2 fires early) --
            with (
                tc.tile_pool(name="s1res", bufs=32) as s1p,
                tc.tile_pool(name="ht", bufs=32) as htp,
            ):
                ht_tiles = {}

                def phase_c_quarter(qq):
                    # s2 rows qq*512 .. +511 (needs ht tiles mb=qq, all f)
                    for mth in range(4):
                        mt = qq * 4 + mth
                        mb, off = mt // 4, (mt % 4) * 128
                        psc = psum.tile([128, OUT], F32, tag="ps", name=f"psC{mt}")
                        for f in range(8):
                            nc.tensor.matmul(
                                psc[:],
                                ht_tiles[(f, mb)][:, off : off + 128],
                                w2t[:, f, :],
                                start=(f == 0),
                                stop=(f == 7),
                            )
                        s2o = smallp.tile([128, OUT], BF16, tag="s2o", bufs=2)
                        nc.vector.tensor_copy(s2o[:], psc[:])
                        nc.scalar.dma_start(
                            ag2_in[qq][mth * 128 : (mth + 1) * 128, :], s2o[:]
                        )
                    allgather(ag2_in[qq], ag2_out[qq])

                for nh in range(2):
                    # s1 sources: nh0 = the gathered half; nh1 = two quarters
                    #   [p,k4,kk,n] layout in all cases
                    if nh == 0:
                        s1_srcs = [
                            ag1h_out[:].rearrange(
                                "(k4 kk p) n -> p k4 kk n", kk=4, p=128
                            )
                        ]
                    else:
                        s1_srcs = [
                            ag1_out[i][:].rearrange(
                                "(k4 kk p) n -> p k4 kk n", kk=4, p=128
                            )
                            for i in range(2)
                        ]
                    s1t = []
                    # m-blocks processed in pairs: each stationary s1 slice
                    # feeds 2 matmuls (adjacent mb), halving LDWEIGHTS count.
                    for mbp in range(2):
                        ps = [
                            psum.tile(
                                [128, 512], F32, tag="ps", name=f"psB{nh}{mbp}{i}"
                            )
                            for i in range(8)
                        ]  # index nt*2 + mbx
                        for k4 in range(32):
                            if mbp == 0:
                                t = s1p.tile(
                                    [128, 4, 512], BF16, tag="s1t",
                                    name=f"s1t{nh}{k4}",
                                )
                                if nh == 0:
                                    nc.sync.dma_start(t[:], s1_srcs[0][:, k4])
                                else:
                                    # two quarter buffers fill column halves
                                    nc.sync.dma_start(
                                        t[:, :, 0:256], s1_srcs[0][:, k4]
                                    )
                                    nc.sync.dma_start(
                                        t[:, :, 256:512], s1_srcs[1][:, k4]
                                    )
                                s1t.append(t)
                            at = adjp.tile(
                                [128, 4, 1024], BF16, tag="adjt", bufs=3,
                                name=f"at{nh}{mbp}{k4}",
                            )
                            nc.sync.dma_start(at[:], adjTp_src(mbp)[:, k4])
                            for kk in range(4):
                                k = k4 * 4 + kk
                                for nt in range(4):
                                    lhs = s1t[k4][:, kk, nt * 128 : (nt + 1) * 128]
                                    for mbx in range(2):
                                        nc.tensor.matmul(
                                            ps[nt * 2 + mbx][:],
                                            lhs,
                                            at[:, kk, mbx * 512 : (mbx + 1) * 512],
                                            start=(k == 0),
                                            stop=(k == 127),
                                        )
                        last_act = None
                        for nt in range(4):
                            j = nh * 4 + nt
                            for mbx in range(2):
                                mb = mbp * 2 + mbx
                                htt = htp.tile([128, 512], BF16, tag="htt")
                                last_act = nc.scalar.activation(
                                    htt[:],
                                    ps[nt * 2 + mbx][:],
                                    mybir.ActivationFunctionType.Relu,
                                    bias=b1t[:, j : j + 1],
                                )
                                ht_tiles[(j, mb)] = htt
                        if nh == 0 and mbp == 0:
                            # fire the second-half s1 gathers now; dep delays
                            # their SDMA traffic past B's startup loads
                            for qi in range(2):
                                cc = allgather(ag1_in[qi], ag1_out[qi])
                                tile_rust_add_dep(
                                    cc.ins,
                                    last_act.ins,
                                    sync=True,
                                    reason="delay s1 q2/q3 gathers past B start",
                                )
                        if nh == 1:
                            # ht tiles for mb 2*mbp..2*mbp+1 now complete for
                            # all f -> emit the matching C quarters + gathers.
                            phase_c_quarter(2 * mbp)
                            phase_c_quarter(2 * mbp + 1)

            # ---- phase D: out2T = (adj_c @ s2)^T + b2 ----
            # All 8 psum banks accumulate concurrently; k-blocks consumed in
            # gather-arrival order (quarter-major), s2 tiles loaded JIT after
            # each adjT chunk so the SP queue stays load-ordered.
            with (
                tc.tile_pool(name="s2res", bufs=32) as s2p,
                tc.tile_pool(name="adjD", bufs=4) as adjDp,
                tc.tile_pool(name="outp", bufs=8) as outp,
            ):
                # ag2_out[qq] rows = g*512 + skk*128 + p  (rank g, block qq)
                s2_srcs = [
                    ag2_out[qq][:].rearrange("(g skk p) n -> p g skk n", g=8, p=128)
                    for qq in range(4)
                ]
                adjD_src = adjT[:].rearrange("(k4 kk p) m -> p k4 kk m", kk=4, p=128)
                dps = [
                    psum.tile([128, 512], F32, tag="ps", name=f"psD{i}")
                    for i in range(8)
                ]
                # k4 = g*4 + qq  ->  iterate quarter-major
                k4_order = [g * 4 + qq for qq in range(4) for g in range(8)]
                for ki, k4 in enumerate(k4_order):
                    g, qq = k4 // 4, k4 % 4
                    at = adjDp.tile([128, 4, SH], BF16, tag="adjD", name=f"atD{k4}")
                    nc.sync.dma_start(at[:], adjD_src[:, k4])
                    st = s2p.tile([128, 4, OUT], BF16, tag="s2t", name=f"s2t{k4}")
                    nc.sync.dma_start(st[:], s2_srcs[qq][:, g])
                    for kk in range(4):
                        for n2t in range(2):
                            lhs = st[:, kk, n2t * 128 : (n2t + 1) * 128]
                            for mb in range(4):
                                nc.tensor.matmul(
                                    dps[n2t * 4 + mb][:],
                                    lhs,
                                    at[:, kk, mb * 512 : (mb + 1) * 512],
                                    start=(ki == 0 and kk == 0),
                                    stop=(ki == 31 and kk == 3),
                                )
                for n2t in range(2):
                    for mb in range(4):
                        ot = outp.tile([128, 512], F32, tag="ot")
                        nc.scalar.activation(
                            ot[:],
                            dps[n2t * 4 + mb][:],
                            mybir.ActivationFunctionType.Identity,
                            bias=b2t[:, n2t : n2t + 1],
                        )
                        nc.scalar.dma_start(
                            out2T[
                                n2t * 128 : (n2t + 1) * 128, mb * 512 : (mb + 1) * 512
                            ],
                            ot[:],
                        )

    _elide_redundant_ldweights(nc)
    _split_excess_waits(nc)
    return nc


def _prep_inputs(x, adj, W1, b1, W2, b2):
    bf = ml_dtypes.bfloat16
    w1b = W1.astype(bf)
    w2b = W2.astype(bf)
    b1T = np.ascontiguousarray(b1.reshape(HID // 128, 128).T).astype(np.float32)
    b2T = np.ascontiguousarray(b2.reshape(OUT // 128, 128).T).astype(np.float32)
    in_maps = []
    for c in range(NCORES):
        rows = slice(c * SH, (c + 1) * SH)
        in_maps.append(
            {
                "adjT": adj[rows, :].T.astype(bf),
                "xT": x[rows, :].T.astype(bf),
                "w1": w1b,
                "w2": w2b,
                "b1T": b1T,
                "b2T": b2T,
            }
        )
    return in_maps


def _run(inputs, trace=False):
    global _built
    if _built is None:
        _built = build()
    in_maps = _prep_inputs(**inputs)
    r = run_bass_kernel_spmd(_built, in_maps, list(range(NCORES)), trace=trace)
    out = np.empty([N, OUT], np.float32)
    for c in range(NCORES):
        out[c * SH : (c + 1) * SH, :] = r.results[c]["out2T"].T
    return out, r


def kernel(x, adj, W1, b1, W2, b2):
    out, _ = _run(dict(x=x, adj=adj, W1=W1, b1=b1, W2=W2, b2=b2))
    return out


# revision 21
# speedup vs baseline: 1.0008x; 1.0008x over previous
"""Trainium2 Bass kernel for a 2-layer dense GCN (NodeEncoder).

    out = adj @ relu(adj @ (x@W1) + b1) @ W2 + b2
    N=16384, F_IN=512, HID=1024, OUT=256, adj dense [N, N] fp32.

Sharding: adj row-partitioned across 8 NeuronCores (2048 rows/core).
All device matmuls consume natural-layout (row-major) operands; the
host pre-transposes adj/x per shard so no on-device transposes are
needed.  Per core:

  phase A:  s1_c   = x_c @ W1                  [2048, 1024]  (own rows)
  AG1:      s1     = AllGather(s1_c)           [16384, 1024]
  phase B:  hT_c   = relu(adj_c @ s1 + b1)^T   [1024, 2048]  (transposed
            orientation: lhsT = s1 tiles, rhs = adjT_c tiles -> psum is
            [n, m]; bias b1 is per-partition, fused into the ACT relu)
  phase C:  s2_c   = h_c @ W2                  [2048, 256]   (lhsT = hT_c)
  AG2:      s2     = AllGather(s2_c)           [16384, 256]
  phase D:  out2T_c = (adj_c @ s2)^T + b2      [256, 2048]   (lhsT = s2
            tiles, rhs = adjT_c tiles; b2 per-partition via ACT Copy)

Matmuls run in bf16 with fp32 PSUM accumulation (max rel err vs fp32
reference ~3e-3 of absmax).
"""

import numpy as np
import ml_dtypes

import concourse.bass as bass
import concourse.mybir as mybir
import concourse.tile as tile
from concourse.bass_utils import run_bass_kernel_spmd
from concourse.tile_sem_assignment import N_PROCS
from concourse.vector_clock import ScopedClock, VectorClock
from concourse.tile_rust import add_dep_helper as tile_rust_add_dep

# ---------------------------------------------------------------------------
# Workaround: the walrus build in this container caps the number of sync-wait
# commands on a Drain instruction; Tile's kernel-tail drain aggregates one
# wait per logical processor and exceeds it.  Split the tail drain into a
# chain of single-wait drains on the same (SP) queue — semantically identical.
# ---------------------------------------------------------------------------


def _drain_and_barrier_split(self, tick_clock, wait_clock):
    gc = tick_clock.global_clock
    for p in range(N_PROCS):
        partial = VectorClock([gc[q] if q == p else 0 for q in range(N_PROCS)])
        d = self.nc.sync.drain()
        wait_clock.add_sem_waits(d.ins, ScopedClock({None: partial}))
    self.nc.sync.drain()

    self.nc.all_engine_barrier()
    assert self.sems is not None
    popped = self.nc._tile_sem_poison_stack.pop()
    assert popped is self._sem_poison
    self.nc.clear_and_free_semaphores(list(self.sems.allocated().values()))
    self.nc.all_engine_barrier()


tile.TileContext._drain_and_barrier = _drain_and_barrier_split

# The same walrus cap applies to every instruction kind: at most ONE sync
# wait command per instruction (probed empirically — a 2-wait TensorCopy is
# rejected).  Post-pass: hoist excess sem-waits onto no-ops inserted just
# before the instruction on the same engine queue — per-engine program order
# makes this semantically identical.
_MAX_WAITS = 1


def _split_excess_waits(nc):
    ctr = 0
    for f in nc.m.functions:
        for bb in f.blocks:
            out = []
            changed = False
            for inst in bb.instructions:
                si = inst.sync_info
                waits = list(si.on_wait) if si is not None and si.on_wait else []
                if len(waits) > _MAX_WAITS:
                    changed = True
                    keep, excess = waits[: _MAX_WAITS], waits[_MAX_WAITS :]
                    for i in range(0, len(excess), _MAX_WAITS):
                        ctr += 1
                        nop = mybir.InstNoOp(name=f"I-waitnop-{ctr}")
                        nop.engine = inst.engine
                        nop.sync_info = mybir.SyncInfo(
                            on_wait=excess[i : i + _MAX_WAITS], on_update=[]
                        )
                        out.append(nop)
                    si.on_wait = keep
                out.append(inst)
            if changed:
                bb.instructions = out
    return ctr

def _elide_redundant_ldweights(nc):
    """Delete an InstLdweights that reloads the exact weights AP loaded by
    the previous (surviving) InstLdweights when only plain matmuls / no-ops
    sit between them in the scheduled stream.  The PE array keeps the
    stationary operand across matmuls, so the reload is pure overhead
    (walrus emits one LDWEIGHTS per MATMUL and its ldw-opt pass is
    incompatible with pre-split LDW+MM).  Only sync-free LDWs are removed,
    so semaphore bookkeeping is unchanged."""
    n_elided = 0
    for f in nc.m.functions:
        for bb in f.blocks:
            out = []
            last_w = None  # weights-AP repr of last surviving LDW, if run intact
            changed = False
            for inst in bb.instructions:
                nm = type(inst).__name__
                if nm == "InstLdweights":
                    si = inst.sync_info
                    clean = not (si and (si.on_wait or si.on_update))
                    w = repr(inst.ins[0])
                    if clean and last_w == w:
                        n_elided += 1
                        changed = True
                        continue  # drop the reload
                    last_w = w if clean else None
                elif nm == "InstMatmult":
                    if getattr(inst, "is_transpose", False):
                        last_w = None
                elif nm == "InstNoOp":
                    pass
                else:
                    last_w = None
                out.append(inst)
            if changed:
                bb.instructions = out
    return n_elided


NCORES = 8
N = 16384
SH = N // NCORES  # 2048 adj rows per core
F = 512
HID = 1024
OUT = 256

BF16 = mybir.dt.bfloat16
F32 = mybir.dt.float32

_built = None


def build():
    """Build the per-core Bass program (identical on all cores)."""
    nc = bass.Bass()

    adjT = nc.declare_dram_parameter("adjT", [N, SH], BF16, isOutput=False)
    xT = nc.declare_dram_parameter("xT", [F, SH], BF16, isOutput=False)
    w1 = nc.declare_dram_parameter("w1", [F, HID], BF16, isOutput=False)
    w2 = nc.declare_dram_parameter("w2", [HID, OUT], BF16, isOutput=False)
    b1T = nc.declare_dram_parameter("b1T", [128, HID // 128], F32, isOutput=False)
    b2T = nc.declare_dram_parameter("b2T", [128, OUT // 128], F32, isOutput=False)
    out2T = nc.declare_dram_parameter("out2T", [OUT, SH], F32, isOutput=True)

    rg = [list(range(NCORES))]

    # adjT column-block mb (512 wide), 4 k-blocks per DMA:
    #   [p, k4, kk, m] = adjT[k4*512 + kk*128 + p, mb*512 + m]
    def adjT_src(mb):
        return adjT[:, mb * 512 : (mb + 1) * 512].rearrange(
            "(k4 kk p) m -> p k4 kk m", kk=4, p=128
        )

    def adjTp_src(mbp):
        return adjT[:, mbp * 1024 : (mbp + 1) * 1024].rearrange(
            "(k4 kk p) m -> p k4 kk m", kk=4, p=128
        )

    def allgather(inp, outp):
        return nc.gpsimd.collective_compute(
            "AllGather",
            mybir.AluOpType.bypass,
            replica_groups=rg,
            ins=[inp.opt()],
            outs=[outp.opt()],
        )

    with tile.TileContext(nc) as tc:
        with (
            tc.tile_pool(name="const", bufs=1) as constp,
            tc.tile_pool(name="psum", bufs=8, space="PSUM") as psum,
            tc.tile_pool(name="dram", bufs=1, space="DRAM") as dram,
            tc.tile_pool(name="adj", bufs=4) as adjp,
            tc.tile_pool(name="small", bufs=4) as smallp,
        ):
            # ---- constants ----
            w2t = constp.tile([128, HID // 128, OUT], BF16)
            nc.sync.dma_start(w2t[:], w2[:].rearrange("(f p) n -> p f n", p=128))
            b1t = constp.tile([128, HID // 128], F32)
            nc.sync.dma_start(b1t[:], b1T[:])
            b2t = constp.tile([128, OUT // 128], F32)
            nc.sync.dma_start(b2t[:], b2T[:])

            # AllGathers split in quarters so they overlap compute: phase B
            # can start once the first two s1 quarters have gathered, and
            # phase D streams k-blocks in gather-arrival order.
            ag1h_in = dram.tile([SH, 512], BF16, name="ag1h_in")
            ag1h_out = dram.tile([N, 512], BF16, addr_space="Shared", name="ag1h_out")
            ag1_in = [dram.tile([SH, 256], BF16, name=f"ag1i{q}") for q in (2, 3)]
            ag1_out = [
                dram.tile([N, 256], BF16, addr_space="Shared", name=f"ag1o{q}")
                for q in (2, 3)
            ]
            ag2_in = [dram.tile([SH // 4, OUT], BF16, name=f"ag2i{q}") for q in range(4)]
            ag2_out = [
                dram.tile([N // 4, OUT], BF16, addr_space="Shared", name=f"ag2o{q}")
                for q in range(4)
            ]

            # ---- phase A: s1_c = x_c @ W1 (per n-quarter; AG per quarter) ----
            with tc.tile_pool(name="phA", bufs=1) as pA:
                xt = []
                w1t = []
                for f in range(4):
                    t = pA.tile([128, SH], BF16, name=f"xt{f}")
                    nc.sync.dma_start(t[:], xT[f * 128 : (f + 1) * 128, :])
                    xt.append(t)
                    t = pA.tile([128, HID], BF16, name=f"w1t{f}")
                    nc.sync.dma_start(t[:], w1[f * 128 : (f + 1) * 128, :])
                    w1t.append(t)
                # first n-half in one chunk: it gates phase B's start
                for mt in range(SH // 128):
                    psa = psum.tile([128, 512], F32, tag="ps", name=f"psAh{mt}")
                    for f in range(4):
                        nc.tensor.matmul(
                            psa[:],
                            xt[f][:, mt * 128 : (mt + 1) * 128],
                            w1t[f][:, 0:512],
                            start=(f == 0),
                            stop=(f == 3),
                        )
                    s1o = smallp.tile([128, 512], BF16, tag="s1o", bufs=2)
                    nc.vector.tensor_copy(s1o[:], psa[:])
                    nc.scalar.dma_start(
                        ag1h_in[mt * 128 : (mt + 1) * 128, :], s1o[:]
                    )
                allgather(ag1h_in, ag1h_out)
                # quarters 2,3: computed now, gathered later (delayed deps)
                for qi, q in enumerate((2, 3)):
                    for mt in range(SH // 128):
                        psa = psum.tile([128, 256], F32, tag="ps", name=f"psA{q}{mt}")
                        for f in range(4):
                            nc.tensor.matmul(
                                psa[:],
                                xt[f][:, mt * 128 : (mt + 1) * 128],
                                w1t[f][:, q * 256 : (q + 1) * 256],
                                start=(f == 0),
                                stop=(f == 3),
                            )
                        s1o2 = smallp.tile([128, 256], BF16, tag="s1o2", bufs=2)
                        nc.vector.tensor_copy(s1o2[:], psa[:])
                        nc.scalar.dma_start(
                            ag1_in[qi][mt * 128 : (mt + 1) * 128, :], s1o2[:]
                        )

            # ---- phases B + C (C quarters interleaved so AG2 fires early) --
            with (
                tc.tile_pool(name="s1res", bufs=32) as s1p,
                tc.tile_pool(name="ht", bufs=32) as htp,
            ):
                ht_tiles = {}

                def phase_c_quarter(qq):
                    # s2 rows qq*512 .. +511 (needs ht tiles mb=qq, all f)
                    for mth in range(4):
                        mt = qq * 4 + mth
                        mb, off = mt // 4, (mt % 4) * 128
                        psc = psum.tile([128, OUT], F32, tag="ps", name=f"psC{mt}")
                        for f in range(8):
                            nc.tensor.matmul(
                                psc[:],
                                ht_tiles[(f, mb)][:, off : off + 128],
                                w2t[:, f, :],
                                start=(f == 0),
                                stop=(f == 7),
                            )
                        s2o = smallp.tile([128, OUT], BF16, tag="s2o", bufs=2)
                        nc.vector.tensor_copy(s2o[:], psc[:])
                        nc.scalar.dma_start(
                            ag2_in[qq][mth * 128 : (mth + 1) * 128, :], s2o[:]
                        )
                    allgather(ag2_in[qq], ag2_out[qq])

                for nh in range(2):
                    # s1 sources: nh0 = the gathered half; nh1 = two quarters
                    #   [p,k4,kk,n] layout in all cases
                    if nh == 0:
                        s1_srcs = [
                            ag1h_out[:].rearrange(
                                "(k4 kk p) n -> p k4 kk n", kk=4, p=128
                            )
                        ]
                    else:
                        s1_srcs = [
                            ag1_out[i][:].rearrange(
                                "(k4 kk p) n -> p k4 kk n", kk=4, p=128
                            )
                            for i in range(2)
                        ]
                    s1t = []
                    # m-blocks processed in pairs: each stationary s1 slice
                    # feeds 2 matmuls (adjacent mb), halving LDWEIGHTS count.
                    for mbp in range(2):
                        ps = [
                            psum.tile(
                                [128, 512], F32, tag="ps", name=f"psB{nh}{mbp}{i}"
                            )
                            for i in range(8)
                        ]  # index nt*2 + mbx
                        for k4 in range(32):
                            if mbp == 0:
                                t = s1p.tile(
                                    [128, 4, 512], BF16, tag="s1t",
                                    name=f"s1t{nh}{k4}",
                                )
                                if nh == 0:
                                    nc.sync.dma_start(t[:], s1_srcs[0][:, k4])
                                else:
                                    # two quarter buffers fill column halves
                                    nc.sync.dma_start(
                                        t[:, :, 0:256], s1_srcs[0][:, k4]
                                    )
                                    nc.sync.dma_start(
                                        t[:, :, 256:512], s1_srcs[1][:, k4]
                                    )
                                s1t.append(t)
                            ats = []
                            for mbx in range(2):
                                atx = adjp.tile(
                                    [128, 4, 512], BF16, tag="adjt", bufs=6,
                                    name=f"at{nh}{mbp}{k4}{mbx}",
                                )
                                nc.sync.dma_start(
                                    atx[:], adjT_src(mbp * 2 + mbx)[:, k4]
                                )
                                ats.append(atx)
                            for kk in range(4):
                                k = k4 * 4 + kk
                                for nt in range(4):
                                    lhs = s1t[k4][:, kk, nt * 128 : (nt + 1) * 128]
                                    for mbx in range(2):
                                        nc.tensor.matmul(
                                            ps[nt * 2 + mbx][:],
                                            lhs,
                                            ats[mbx][:, kk, :],
                                            start=(k == 0),
                                            stop=(k == 127),
                                        )
                        last_act = None
                        for nt in range(4):
                            j = nh * 4 + nt
                            for mbx in range(2):
                                mb = mbp * 2 + mbx
                                htt = htp.tile([128, 512], BF16, tag="htt")
                                last_act = nc.scalar.activation(
                                    htt[:],
                                    ps[nt * 2 + mbx][:],
                                    mybir.ActivationFunctionType.Relu,
                                    bias=b1t[:, j : j + 1],
                                )
                                ht_tiles[(j, mb)] = htt
                        if nh == 0 and mbp == 0:
                            # fire the second-half s1 gathers now; dep delays
                            # their SDMA traffic past B's startup loads
                            for qi in range(2):
                                cc = allgather(ag1_in[qi], ag1_out[qi])
                                tile_rust_add_dep(
                                    cc.ins,
                                    last_act.ins,
                                    sync=True,
                                    reason="delay s1 q2/q3 gathers past B start",
                                )
                        if nh == 1:
                            # ht tiles for mb 2*mbp..2*mbp+1 now complete for
                            # all f -> emit the matching C quarters + gathers.
                            phase_c_quarter(2 * mbp)
                            phase_c_quarter(2 * mbp + 1)

            # ---- phase D: out2T = (adj_c @ s2)^T + b2 ----
            # All 8 psum banks accumulate concurrently; k-blocks consumed in
            # gather-arrival order (quarter-major), s2 tiles loaded JIT after
            # each adjT chunk so the SP queue stays load-ordered.
            with (
                tc.tile_pool(name="s2res", bufs=32) as s2p,
                tc.tile_pool(name="adjD", bufs=4) as adjDp,
                tc.tile_pool(name="outp", bufs=8) as outp,
            ):
                # ag2_out[qq] rows = g*512 + skk*128 + p  (rank g, block qq)
                s2_srcs = [
                    ag2_out[qq][:].rearrange("(g skk p) n -> p g skk n", g=8, p=128)
                    for qq in range(4)
                ]
                adjD_src = adjT[:].rearrange("(k4 kk p) m -> p k4 kk m", kk=4, p=128)
                dps = [
                    psum.tile([128, 512], F32, tag="ps", name=f"psD{i}")
                    for i in range(8)
                ]
                # k4 = g*4 + qq  ->  iterate quarter-major
                k4_order = [g * 4 + qq for qq in range(4) for g in range(8)]
                for ki, k4 in enumerate(k4_order):
                    g, qq = k4 // 4, k4 % 4
                    at = adjDp.tile([128, 4, SH], BF16, tag="adjD", name=f"atD{k4}")
                    nc.sync.dma_start(at[:], adjD_src[:, k4])
                    st = s2p.tile([128, 4, OUT], BF16, tag="s2t", name=f"s2t{k4}")
                    nc.sync.dma_start(st[:], s2_srcs[qq][:, g])
                    for kk in range(4):
                        for n2t in range(2):
                            lhs = st[:, kk, n2t * 128 : (n2t + 1) * 128]
                            for mb in range(4):
                                nc.tensor.matmul(
                                    dps[n2t * 4 + mb][:],
                                    lhs,
                                    at[:, kk, mb * 512 : (mb + 1) * 512],
                                    start=(ki == 0 and kk == 0),
                                    stop=(ki == 31 and kk == 3),
                                )
                for n2t in range(2):
                    for mb in range(4):
                        ot = outp.tile([128, 512], F32, tag="ot")
                        nc.scalar.activation(
                            ot[:],
                            dps[n2t * 4 + mb][:],
                            mybir.ActivationFunctionType.Identity,
                            bias=b2t[:, n2t : n2t + 1],
                        )
                        nc.scalar.dma_start(
                            out2T[
                                n2t * 128 : (n2t + 1) * 128, mb * 512 : (mb + 1) * 512
                            ],
                            ot[:],
                        )

    _elide_redundant_ldweights(nc)
    _split_excess_waits(nc)
    return nc


def _prep_inputs(x, adj, W1, b1, W2, b2):
    bf = ml_dtypes.bfloat16
    w1b = W1.astype(bf)
    w2b = W2.astype(bf)
    b1T = np.ascontiguousarray(b1.reshape(HID // 128, 128).T).astype(np.float32)
    b2T = np.ascontiguousarray(b2.reshape(OUT // 128, 128).T).astype(np.float32)
    in_maps = []
    for c in range(NCORES):
        rows = slice(c * SH, (c + 1) * SH)
        in_maps.append(
            {
                "adjT": adj[rows, :].T.astype(bf),
                "xT": x[rows, :].T.astype(bf),
                "w1": w1b,
                "w2": w2b,
                "b1T": b1T,
                "b2T": b2T,
            }
        )
    return in_maps


def _run(inputs, trace=False):
    global _built
    if _built is None:
        _built = build()
    in_maps = _prep_inputs(**inputs)
    r = run_bass_kernel_spmd(_built, in_maps, list(range(NCORES)), trace=trace)
    out = np.empty([N, OUT], np.float32)
    for c in range(NCORES):
        out[c * SH : (c + 1) * SH, :] = r.results[c]["out2T"].T
    return out, r


def kernel(x, adj, W1, b1, W2, b2):
    out, _ = _run(dict(x=x, adj=adj, W1=W1, b1=b1, W2=W2, b2=b2))
    return out


# revision 22
# speedup vs baseline: 1.0057x; 1.0049x over previous
"""Trainium2 Bass kernel for a 2-layer dense GCN (NodeEncoder).

    out = adj @ relu(adj @ (x@W1) + b1) @ W2 + b2
    N=16384, F_IN=512, HID=1024, OUT=256, adj dense [N, N] fp32.

Sharding: adj row-partitioned across 8 NeuronCores (2048 rows/core).
All device matmuls consume natural-layout (row-major) operands; the
host pre-transposes adj/x per shard so no on-device transposes are
needed.  Per core:

  phase A:  s1_c   = x_c @ W1                  [2048, 1024]  (own rows)
  AG1:      s1     = AllGather(s1_c)           [16384, 1024]
  phase B:  hT_c   = relu(adj_c @ s1 + b1)^T   [1024, 2048]  (transposed
            orientation: lhsT = s1 tiles, rhs = adjT_c tiles -> psum is
            [n, m]; bias b1 is per-partition, fused into the ACT relu)
  phase C:  s2_c   = h_c @ W2                  [2048, 256]   (lhsT = hT_c)
  AG2:      s2     = AllGather(s2_c)           [16384, 256]
  phase D:  out2T_c = (adj_c @ s2)^T + b2      [256, 2048]   (lhsT = s2
            tiles, rhs = adjT_c tiles; b2 per-partition via ACT Copy)

Matmuls run in bf16 with fp32 PSUM accumulation (max rel err vs fp32
reference ~3e-3 of absmax).
"""

import numpy as np
import ml_dtypes

import concourse.bass as bass
import concourse.mybir as mybir
import concourse.tile as tile
from concourse.bass_utils import run_bass_kernel_spmd
from concourse.tile_sem_assignment import N_PROCS
from concourse.vector_clock import ScopedClock, VectorClock
from concourse.tile_rust import add_dep_helper as tile_rust_add_dep

# ---------------------------------------------------------------------------
# Workaround: the walrus build in this container caps the number of sync-wait
# commands on a Drain instruction; Tile's kernel-tail drain aggregates one
# wait per logical processor and exceeds it.  Split the tail drain into a
# chain of single-wait drains on the same (SP) queue — semantically identical.
# ---------------------------------------------------------------------------


def _drain_and_barrier_split(self, tick_clock, wait_clock):
    gc = tick_clock.global_clock
    for p in range(N_PROCS):
        partial = VectorClock([gc[q] if q == p else 0 for q in range(N_PROCS)])
        d = self.nc.sync.drain()
        wait_clock.add_sem_waits(d.ins, ScopedClock({None: partial}))
    self.nc.sync.drain()

    self.nc.all_engine_barrier()
    assert self.sems is not None
    popped = self.nc._tile_sem_poison_stack.pop()
    assert popped is self._sem_poison
    self.nc.clear_and_free_semaphores(list(self.sems.allocated().values()))
    self.nc.all_engine_barrier()


tile.TileContext._drain_and_barrier = _drain_and_barrier_split

# The same walrus cap applies to every instruction kind: at most ONE sync
# wait command per instruction (probed empirically — a 2-wait TensorCopy is
# rejected).  Post-pass: hoist excess sem-waits onto no-ops inserted just
# before the instruction on the same engine queue — per-engine program order
# makes this semantically identical.
_MAX_WAITS = 1


def _split_excess_waits(nc):
    ctr = 0
    for f in nc.m.functions:
        for bb in f.blocks:
            out = []
            changed = False
            for inst in bb.instructions:
                si = inst.sync_info
                waits = list(si.on_wait) if si is not None and si.on_wait else []
                if len(waits) > _MAX_WAITS:
                    changed = True
                    keep, excess = waits[: _MAX_WAITS], waits[_MAX_WAITS :]
                    for i in range(0, len(excess), _MAX_WAITS):
                        ctr += 1
                        nop = mybir.InstNoOp(name=f"I-waitnop-{ctr}")
                        nop.engine = inst.engine
                        nop.sync_info = mybir.SyncInfo(
                            on_wait=excess[i : i + _MAX_WAITS], on_update=[]
                        )
                        out.append(nop)
                    si.on_wait = keep
                out.append(inst)
            if changed:
                bb.instructions = out
    return ctr

def _elide_redundant_ldweights(nc):
    """Delete an InstLdweights that reloads the exact weights AP loaded by
    the previous (surviving) InstLdweights when only plain matmuls / no-ops
    sit between them in the scheduled stream.  The PE array keeps the
    stationary operand across matmuls, so the reload is pure overhead
    (walrus emits one LDWEIGHTS per MATMUL and its ldw-opt pass is
    incompatible with pre-split LDW+MM).  Only sync-free LDWs are removed,
    so semaphore bookkeeping is unchanged."""
    n_elided = 0
    for f in nc.m.functions:
        for bb in f.blocks:
            out = []
            last_w = None  # weights-AP repr of last surviving LDW, if run intact
            changed = False
            for inst in bb.instructions:
                nm = type(inst).__name__
                if nm == "InstLdweights":
                    si = inst.sync_info
                    clean = not (si and (si.on_wait or si.on_update))
                    w = repr(inst.ins[0])
                    if clean and last_w == w:
                        n_elided += 1
                        changed = True
                        continue  # drop the reload
                    last_w = w if clean else None
                elif nm == "InstMatmult":
                    if getattr(inst, "is_transpose", False):
                        last_w = None
                elif nm == "InstNoOp":
                    pass
                else:
                    last_w = None
                out.append(inst)
            if changed:
                bb.instructions = out
    return n_elided


NCORES = 8
N = 16384
SH = N // NCORES  # 2048 adj rows per core
F = 512
HID = 1024
OUT = 256

BF16 = mybir.dt.bfloat16
F32 = mybir.dt.float32

_built = None


def build():
    """Build the per-core Bass program (identical on all cores)."""
    nc = bass.Bass()

    adjT = nc.declare_dram_parameter("adjT", [N, SH], BF16, isOutput=False)
    xT = nc.declare_dram_parameter("xT", [F, SH], BF16, isOutput=False)
    w1 = nc.declare_dram_parameter("w1", [F, HID], BF16, isOutput=False)
    w2 = nc.declare_dram_parameter("w2", [HID, OUT], BF16, isOutput=False)
    b1T = nc.declare_dram_parameter("b1T", [128, HID // 128], F32, isOutput=False)
    b2T = nc.declare_dram_parameter("b2T", [128, OUT // 128], F32, isOutput=False)
    out2T = nc.declare_dram_parameter("out2T", [OUT, SH], F32, isOutput=True)

    rg = [list(range(NCORES))]

    # adjT column-block mb (512 wide), 4 k-blocks per DMA:
    #   [p, k4, kk, m] = adjT[k4*512 + kk*128 + p, mb*512 + m]
    def adjT_src(mb):
        return adjT[:, mb * 512 : (mb + 1) * 512].rearrange(
            "(k4 kk p) m -> p k4 kk m", kk=4, p=128
        )

    def adjTp_src(mbp):
        return adjT[:, mbp * 1024 : (mbp + 1) * 1024].rearrange(
            "(k4 kk p) m -> p k4 kk m", kk=4, p=128
        )

    def allgather(inp, outp):
        return nc.gpsimd.collective_compute(
            "AllGather",
            mybir.AluOpType.bypass,
            replica_groups=rg,
            ins=[inp.opt()],
            outs=[outp.opt()],
        )

    with tile.TileContext(nc) as tc:
        with (
            tc.tile_pool(name="const", bufs=1) as constp,
            tc.tile_pool(name="psum", bufs=8, space="PSUM") as psum,
            tc.tile_pool(name="dram", bufs=1, space="DRAM") as dram,
            tc.tile_pool(name="adj", bufs=4) as adjp,
            tc.tile_pool(name="small", bufs=4) as smallp,
        ):
            # ---- constants ----
            w2t = constp.tile([128, HID // 128, OUT], BF16)
            nc.sync.dma_start(w2t[:], w2[:].rearrange("(f p) n -> p f n", p=128))
            b1t = constp.tile([128, HID // 128], F32)
            nc.sync.dma_start(b1t[:], b1T[:])
            b2t = constp.tile([128, OUT // 128], F32)
            nc.sync.dma_start(b2t[:], b2T[:])

            # AllGathers split in quarters so they overlap compute: phase B
            # can start once the first two s1 quarters have gathered, and
            # phase D streams k-blocks in gather-arrival order.
            ag1h_in = dram.tile([SH, 512], BF16, name="ag1h_in")
            ag1h_out = dram.tile([N, 512], BF16, addr_space="Shared", name="ag1h_out")
            ag1_in = [dram.tile([SH, 256], BF16, name=f"ag1i{q}") for q in (2, 3)]
            ag1_out = [
                dram.tile([N, 256], BF16, addr_space="Shared", name=f"ag1o{q}")
                for q in (2, 3)
            ]
            ag2_in = [dram.tile([SH // 4, OUT], BF16, name=f"ag2i{q}") for q in range(4)]
            ag2_out = [
                dram.tile([N // 4, OUT], BF16, addr_space="Shared", name=f"ag2o{q}")
                for q in range(4)
            ]

            # ---- phase A: s1_c = x_c @ W1 (per n-quarter; AG per quarter) ----
            with tc.tile_pool(name="phA", bufs=1) as pA:
                xt = []
                w1t = []
                for f in range(4):
                    t = pA.tile([128, SH], BF16, name=f"xt{f}")
                    nc.sync.dma_start(t[:], xT[f * 128 : (f + 1) * 128, :])
                    xt.append(t)
                    t = pA.tile([128, HID], BF16, name=f"w1t{f}")
                    nc.sync.dma_start(t[:], w1[f * 128 : (f + 1) * 128, :])
                    w1t.append(t)
                # first n-half in one chunk: it gates phase B's start
                for mt in range(SH // 128):
                    psa = psum.tile([128, 512], F32, tag="ps", name=f"psAh{mt}")
                    for f in range(4):
                        nc.tensor.matmul(
                            psa[:],
                            xt[f][:, mt * 128 : (mt + 1) * 128],
                            w1t[f][:, 0:512],
                            start=(f == 0),
                            stop=(f == 3),
                        )
                    s1o = smallp.tile([128, 512], BF16, tag="s1o", bufs=2)
                    nc.vector.tensor_copy(s1o[:], psa[:])
                    nc.scalar.dma_start(
                        ag1h_in[mt * 128 : (mt + 1) * 128, :], s1o[:]
                    )
                allgather(ag1h_in, ag1h_out)
                # quarters 2,3: computed now, gathered later (delayed deps)
                for qi, q in enumerate((2, 3)):
                    for mt in range(SH // 128):
                        psa = psum.tile([128, 256], F32, tag="ps", name=f"psA{q}{mt}")
                        for f in range(4):
                            nc.tensor.matmul(
                                psa[:],
                                xt[f][:, mt * 128 : (mt + 1) * 128],
                                w1t[f][:, q * 256 : (q + 1) * 256],
                                start=(f == 0),
                                stop=(f == 3),
                            )
                        s1o2 = smallp.tile([128, 256], BF16, tag="s1o2", bufs=2)
                        nc.vector.tensor_copy(s1o2[:], psa[:])
                        nc.scalar.dma_start(
                            ag1_in[qi][mt * 128 : (mt + 1) * 128, :], s1o2[:]
                        )

            # ---- phases B + C (C quarters interleaved so AG2 fires early) --
            with (
                tc.tile_pool(name="s1res", bufs=32) as s1p,
                tc.tile_pool(name="ht", bufs=32) as htp,
            ):
                ht_tiles = {}

                def phase_c_quarter(qq):
                    # s2 rows qq*512 .. +511 (needs ht tiles mb=qq, all f)
                    for mth in range(4):
                        mt = qq * 4 + mth
                        mb, off = mt // 4, (mt % 4) * 128
                        psc = psum.tile([128, OUT], F32, tag="ps", name=f"psC{mt}")
                        for f in range(8):
                            nc.tensor.matmul(
                                psc[:],
                                ht_tiles[(f, mb)][:, off : off + 128],
                                w2t[:, f, :],
                                start=(f == 0),
                                stop=(f == 7),
                            )
                        s2o = smallp.tile([128, OUT], BF16, tag="s2o", bufs=2)
                        nc.vector.tensor_copy(s2o[:], psc[:])
                        nc.scalar.dma_start(
                            ag2_in[qq][mth * 128 : (mth + 1) * 128, :], s2o[:]
                        )
                    allgather(ag2_in[qq], ag2_out[qq])

                for nh in range(2):
                    # s1 sources: nh0 = the gathered half; nh1 = two quarters
                    #   [p,k4,kk,n] layout in all cases
                    if nh == 0:
                        s1_srcs = [
                            ag1h_out[:].rearrange(
                                "(k4 kk p) n -> p k4 kk n", kk=4, p=128
                            )
                        ]
                    else:
                        s1_srcs = [
                            ag1_out[i][:].rearrange(
                                "(k4 kk p) n -> p k4 kk n", kk=4, p=128
                            )
                            for i in range(2)
                        ]
                    s1t = []
                    # m-blocks processed in pairs: each stationary s1 slice
                    # feeds 2 matmuls (adjacent mb), halving LDWEIGHTS count.
                    for mbp in range(2):
                        ps = [
                            psum.tile(
                                [128, 512], F32, tag="ps", name=f"psB{nh}{mbp}{i}"
                            )
                            for i in range(8)
                        ]  # index nt*2 + mbx
                        for k4 in range(32):
                            if mbp == 0:
                                t = s1p.tile(
                                    [128, 4, 512], BF16, tag="s1t",
                                    name=f"s1t{nh}{k4}",
                                )
                                if nh == 0:
                                    nc.sync.dma_start(t[:], s1_srcs[0][:, k4])
                                else:
                                    # two quarter buffers fill column halves
                                    nc.sync.dma_start(
                                        t[:, :, 0:256], s1_srcs[0][:, k4]
                                    )
                                    nc.sync.dma_start(
                                        t[:, :, 256:512], s1_srcs[1][:, k4]
                                    )
                                s1t.append(t)
                            ats = []
                            for mbx in range(2):
                                atx = adjp.tile(
                                    [128, 4, 512], BF16, tag="adjt", bufs=4,
                                    name=f"at{nh}{mbp}{k4}{mbx}",
                                )
                                nc.sync.dma_start(
                                    atx[:], adjT_src(mbp * 2 + mbx)[:, k4]
                                )
                                ats.append(atx)
                            for kk in range(4):
                                k = k4 * 4 + kk
                                for nt in range(4):
                                    lhs = s1t[k4][:, kk, nt * 128 : (nt + 1) * 128]
                                    for mbx in range(2):
                                        nc.tensor.matmul(
                                            ps[nt * 2 + mbx][:],
                                            lhs,
                                            ats[mbx][:, kk, :],
                                            start=(k == 0),
                                            stop=(k == 127),
                                        )
                        last_act = None
                        for nt in range(4):
                            j = nh * 4 + nt
                            for mbx in range(2):
                                mb = mbp * 2 + mbx
                                htt = htp.tile([128, 512], BF16, tag="htt")
                                last_act = nc.scalar.activation(
                                    htt[:],
                                    ps[nt * 2 + mbx][:],
                                    mybir.ActivationFunctionType.Relu,
                                    bias=b1t[:, j : j + 1],
                                )
                                ht_tiles[(j, mb)] = htt
                        if nh == 0 and mbp == 0:
                            # fire the second-half s1 gathers now; dep delays
                            # their SDMA traffic past B's startup loads
                            for qi in range(2):
                                cc = allgather(ag1_in[qi], ag1_out[qi])
                                tile_rust_add_dep(
                                    cc.ins,
                                    last_act.ins,
                                    sync=True,
                                    reason="delay s1 q2/q3 gathers past B start",
                                )
                        if nh == 1:
                            # ht tiles for mb 2*mbp..2*mbp+1 now complete for
                            # all f -> emit the matching C quarters + gathers.
                            phase_c_quarter(2 * mbp)
                            phase_c_quarter(2 * mbp + 1)

            # ---- phase D: out2T = (adj_c @ s2)^T + b2 ----
            # All 8 psum banks accumulate concurrently; k-blocks consumed in
            # gather-arrival order (quarter-major), s2 tiles loaded JIT after
            # each adjT chunk so the SP queue stays load-ordered.
            with (
                tc.tile_pool(name="s2res", bufs=32) as s2p,
                tc.tile_pool(name="adjD", bufs=4) as adjDp,
                tc.tile_pool(name="outp", bufs=8) as outp,
            ):
                # ag2_out[qq] rows = g*512 + skk*128 + p  (rank g, block qq)
                s2_srcs = [
                    ag2_out[qq][:].rearrange("(g skk p) n -> p g skk n", g=8, p=128)
                    for qq in range(4)
                ]
                adjD_src = adjT[:].rearrange("(k4 kk p) m -> p k4 kk m", kk=4, p=128)
                dps = [
                    psum.tile([128, 512], F32, tag="ps", name=f"psD{i}")
                    for i in range(8)
                ]
                # k4 = g*4 + qq  ->  iterate quarter-major
                k4_order = [g * 4 + qq for qq in range(4) for g in range(8)]
                for ki, k4 in enumerate(k4_order):
                    g, qq = k4 // 4, k4 % 4
                    at = adjDp.tile([128, 4, SH], BF16, tag="adjD", name=f"atD{k4}")
                    nc.sync.dma_start(at[:], adjD_src[:, k4])
                    st = s2p.tile([128, 4, OUT], BF16, tag="s2t", name=f"s2t{k4}")
                    nc.sync.dma_start(st[:], s2_srcs[qq][:, g])
                    for kk in range(4):
                        for n2t in range(2):
                            lhs = st[:, kk, n2t * 128 : (n2t + 1) * 128]
                            for mb in range(4):
                                nc.tensor.matmul(
                                    dps[n2t * 4 + mb][:],
                                    lhs,
                                    at[:, kk, mb * 512 : (mb + 1) * 512],
                                    start=(ki == 0 and kk == 0),
                                    stop=(ki == 31 and kk == 3),
                                )
                for n2t in range(2):
                    for mb in range(4):
                        ot = outp.tile([128, 512], F32, tag="ot")
                        nc.scalar.activation(
                            ot[:],
                            dps[n2t * 4 + mb][:],
                            mybir.ActivationFunctionType.Identity,
                            bias=b2t[:, n2t : n2t + 1],
                        )
                        nc.scalar.dma_start(
                            out2T[
                                n2t * 128 : (n2t + 1) * 128, mb * 512 : (mb + 1) * 512
                            ],
                            ot[:],
                        )

    _elide_redundant_ldweights(nc)
    _split_excess_waits(nc)
    return nc


def _prep_inputs(x, adj, W1, b1, W2, b2):
    bf = ml_dtypes.bfloat16
    w1b = W1.astype(bf)
    w2b = W2.astype(bf)
    b1T = np.ascontiguousarray(b1.reshape(HID // 128, 128).T).astype(np.float32)
    b2T = np.ascontiguousarray(b2.reshape(OUT // 128, 128).T).astype(np.float32)
    in_maps = []
    for c in range(NCORES):
        rows = slice(c * SH, (c + 1) * SH)
        in_maps.append(
            {
                "adjT": adj[rows, :].T.astype(bf),
                "xT": x[rows, :].T.astype(bf),
                "w1": w1b,
                "w2": w2b,
                "b1T": b1T,
                "b2T": b2T,
            }
        )
    return in_maps


def _run(inputs, trace=False):
    global _built
    if _built is None:
        _built = build()
    in_maps = _prep_inputs(**inputs)
    r = run_bass_kernel_spmd(_built, in_maps, list(range(NCORES)), trace=trace)
    out = np.empty([N, OUT], np.float32)
    for c in range(NCORES):
        out[c * SH : (c + 1) * SH, :] = r.results[c]["out2T"].T
    return out, r


def kernel(x, adj, W1, b1, W2, b2):
    out, _ = _run(dict(x=x, adj=adj, W1=W1, b1=b1, W2=W2, b2=b2))
    return out
